# revision 11
# baseline (speedup 1.0000x reference)
"""GCN layer on 8 Trainium2 NeuronCores.

Computes relu(D^-1/2 (A+I) D^-1/2 X W + b) for N=8192, d=256.

Sharding: row-shard adj over N across the 8 cores (1024 rows each); x, W, b
replicated. Each core's shard is laid out column-major (adj[rows].T) and in
bf16 (the tensor-engine compute precision) so the contraction dim (adj
columns j) lands on SBUF partitions, which the PE matmul requires.

Pipeline per core (single NEFF):
  1. Stream the 16MB shard once (HWDGE) into a persistent SBUF cache, in two
     stages: first the columns for its rows 0:512 ("A"), then rows 512:1024
     ("B"). The tensor engine reduces row sums (matmul vs ones) as tiles land.
  2. AllGather #1 ships the A-half degrees while the B-half still streams;
     AllGather #2 ships the B-half. Degrees come back through a natural
     (contiguous) DMA + PE transpose into per-partition D^-1/2 tables.
  3. U^T = ((A+I) y)^T with y = D^-1/2 x: y is scaled chunk-by-chunk
     (alternating Scalar/Vector engines) just ahead of the matmuls; the
     matmuls for row-half A start after AllGather #1, hiding AllGather #2;
     +I enters via identity-matmuls of the core's own y rows.
  4. Scale by own D^-1/2 (free-dim broadcast), apply W, bias, ReLU, and
     write the output block transposed; the host stitches the 8 blocks.
"""

import numpy as np

N = 8192
D = 256
NCORES = 8
R = N // NCORES  # rows per core = 1024
KT = N // 128  # 64 j-tiles
TS = R // 128  # 8 own-row tiles

_CACHE = {}


def _build_nc():
    import concourse.bacc as bacc
    import concourse.tile as tile
    import concourse.mybir as mybir

    f32 = mybir.dt.float32
    bf16 = mybir.dt.bfloat16
    AF = mybir.ActivationFunctionType

    nc = bacc.Bacc("TRN2", target_bir_lowering=False, debug=False,
                   num_devices=NCORES)

    adjT = nc.dram_tensor("adjT", [N, R], bf16, kind="ExternalInput")
    xin = nc.dram_tensor("x", [N, D], bf16, kind="ExternalInput")
    xown = nc.dram_tensor("xown", [R, D], bf16, kind="ExternalInput")
    Win = nc.dram_tensor("W", [D, D], bf16, kind="ExternalInput")
    bin_ = nc.dram_tensor("b", [D], f32, kind="ExternalInput")
    eyeb = nc.dram_tensor("eye", [128, 128], bf16, kind="ExternalInput")
    eyef = nc.dram_tensor("eyef", [128, 128], f32, kind="ExternalInput")
    outT = nc.dram_tensor("outT", [D, R], f32, kind="ExternalOutput")
    warmo = nc.dram_tensor("warmo", [128], f32, kind="ExternalOutput")

    with tile.TileContext(nc) as tc:
        from contextlib import ExitStack

        with ExitStack() as ctx:
            pp = ctx.enter_context(tc.tile_pool(name="persist", bufs=1))
            dp = ctx.enter_context(tc.tile_pool(name="dram", bufs=1, space="DRAM"))

            # ---- persistent SBUF tensors ----
            adjTb = pp.tile([128, KT * R], bf16)   # 128KB/partition cache
            xb = pp.tile([128, KT * D], bf16)      # x, partition = j%128
            xob = pp.tile([128, TS * D], bf16)     # own x rows
            Wb = pp.tile([128, 2 * D], bf16)       # W, partition = n%128
            bsb = pp.tile([128, 2], f32)           # bias, partition = m%128
            eye_s = pp.tile([128, 128], bf16)
            eyef_s = pp.tile([128, 128], f32)
            ones_bf = pp.tile([128, 1], bf16)
            deg_s = pp.tile([1, R], f32)           # local degree (+1), A|B halves
            disl = pp.tile([1, R], f32)            # local D^-1/2
            degn = pp.tile([64, 128], f32)         # gathered degrees, natural
            degln = pp.tile([8, 128], f32)         # local degrees, natural
            dis_pp = pp.tile([128, 64], f32)       # D^-1/2, partition = j%128
            diso = pp.tile([128, TS], f32)         # own D^-1/2, partition = i%128
            disrep = pp.tile([128, R], f32)        # own D^-1/2 on free dim
            y2 = [pp.tile([128, R], bf16, name=f"y2_{i}") for i in range(2)]
            outsb = [pp.tile([128, R], f32, name=f"outsb_{i}") for i in range(2)]

            degl_d = dp.tile([R], f32)
            dega_d = dp.tile([N], f32)
            disl_d = dp.tile([R], f32)

            nc.any.memset(ones_bf[:], 1.0)

            # ---- small loads first (sync HWDGE queue) ----
            nc.sync.dma_start(
                out=xob[:, :].rearrange("p (t n) -> p t n", n=D),
                in_=xown.ap().rearrange("(t p) n -> p t n", p=128))
            nc.sync.dma_start(
                out=Wb[:, :].rearrange("p (k m) -> p k m", m=D),
                in_=Win.ap().rearrange("(k p) m -> p k m", p=128))
            nc.sync.dma_start(
                out=bsb[:, :], in_=bin_.ap().rearrange("(h p) -> p h", p=128))
            nc.sync.dma_start(out=eye_s[:, :], in_=eyeb.ap())
            nc.sync.dma_start(out=eyef_s[:, :], in_=eyef.ap())

            # ---- dummy collective: absorbs the ~60us ncfw cold start ----
            warm_sb = pp.tile([1, 128], f32)
            nc.any.memset(warm_sb[:], 1.0)
            warm_in = dp.tile([128], f32)
            warm_out = dp.tile([NCORES * 128], f32)
            nc.scalar.dma_start(out=warm_in[:], in_=warm_sb[0:1, :])
            nc.gpsimd.collective_compute(
                "AllGather", mybir.AluOpType.bypass,
                replica_groups=[list(range(NCORES))],
                ins=[warm_in.opt()], outs=[warm_out.opt()])
            nc.scalar.dma_start(out=warmo.ap(), in_=warm_out.opt()[0:128])

            # ---- phase 1: stream adjT + row sums ----
            GC = 8  # j-tiles per DMA chunk
            adjTb3 = adjTb[:, :].rearrange("p (k i) -> p k i", i=R)
            xb3 = xb[:, :].rearrange("p (k n) -> p k n", n=D)
            xin3 = xin.ap().rearrange("(k p) n -> p k n", p=128)

            for g in range(KT // GC):
                src = adjT.ap()[g * GC * 128:(g + 1) * GC * 128,
                                :].rearrange("(k p) i -> p k i", p=128)
                nc.sync.dma_start(
                    out=adjTb3[:, g * GC:(g + 1) * GC, :], in_=src)
            nc.sync.dma_start(out=xb3[:, :, :], in_=xin3[:, :, :])

            pdeg = ctx.enter_context(tc.tile_pool(name="psdeg", bufs=1, space="PSUM"))
            pst = ctx.enter_context(tc.tile_pool(name="pst", bufs=1, space="PSUM"))
            psuo = ctx.enter_context(tc.tile_pool(name="psuo", bufs=2, space="PSUM"))

            dps = pdeg.tile([1, 1024], f32, padded_shape=[128, 1024])
            for s in range(2):
                for k in range(KT):
                    nc.tensor.matmul(
                        dps[:, s * 512:(s + 1) * 512], ones_bf[:, :],
                        adjTb[:, k * R + s * 512:k * R + (s + 1) * 512],
                        start=(k == 0), stop=(k == KT - 1),
                        skip_group_check=True)
                # deg = rowsum + 1 (the +I term)
                nc.vector.tensor_scalar_add(
                    deg_s[:, s * 512:(s + 1) * 512],
                    dps[:, s * 512:(s + 1) * 512], 1.0)
            nc.scalar.dma_start(out=degl_d[:], in_=deg_s[0:1, :])
            nc.gpsimd.collective_compute(
                "AllGather", mybir.AluOpType.bypass,
                replica_groups=[list(range(NCORES))],
                ins=[degl_d.opt()], outs=[dega_d.opt()])

            # local dis for the free-dim broadcast (via DRAM round trip)
            nc.vector.reciprocal_approx_fast(disl[:, :], deg_s[:, :])
            nc.scalar.activation(disl[:, :], disl[:, :], AF.Sqrt)
            nc.scalar.dma_start(out=disl_d[:], in_=disl[0:1, :])
            nc.scalar.dma_start(
                out=disrep[:, :],
                in_=disl_d.opt().unsqueeze(0).partition_broadcast(128))

            # gathered degrees -> per-partition D^-1/2 via PE transpose
            tall = pst.tile([128, 72], f32)
            nc.scalar.dma_start(
                out=degn[:, :], in_=dega_d.opt().rearrange("(c f) -> c f", f=128))
            nc.tensor.transpose(tall[:, 0:64], degn[:, :], eyef_s[0:64, 0:64])
            nc.vector.reciprocal_approx_fast(dis_pp[:, :], tall[:, 0:64])
            nc.scalar.activation(dis_pp[:, :], dis_pp[:, :], AF.Sqrt)
            # local degrees -> own D^-1/2 table (for the +I rows)
            nc.scalar.dma_start(
                out=degln[:, :], in_=degl_d.opt().rearrange("(c f) -> c f", f=128))
            nc.tensor.transpose(tall[:, 64:72], degln[:, :], eyef_s[0:8, 0:8])
            nc.vector.reciprocal_approx_fast(diso[:, :], tall[:, 64:72])
            nc.scalar.activation(diso[:, :], diso[:, :], AF.Sqrt)

            # ---- phase 3: y = dis*x and U^T = ((A+I) y)^T ----
            u = [psuo.tile([128, R], f32, name=f"u_{i}", tag="uo") for i in range(2)]

            def dis_col(k):
                return dis_pp[:, k:k + 1]

            def scale_y(k):
                chunk = xb[:, k * D:(k + 1) * D]
                if k % 2 == 0:
                    nc.scalar.activation(chunk, chunk, AF.Copy,
                                         scale=dis_col(k))
                else:
                    nc.vector.tensor_scalar_mul(chunk, chunk, dis_col(k))

            for k in range(KT):
                scale_y(k)
                for h in range(2):
                    for s in range(2):
                        nc.tensor.matmul(
                            u[h][:, s * 512:(s + 1) * 512],
                            xb[:, k * D + h * 128:k * D + (h + 1) * 128],
                            adjTb[:, k * R + s * 512:k * R + (s + 1) * 512],
                            start=(k == 0), stop=False,
                            skip_group_check=True)
            # +I: U^T[n, own block t] += y_own[t]^T
            for t in range(TS):
                chunk = xob[:, t * D:(t + 1) * D]
                nc.scalar.activation(chunk, chunk, AF.Copy,
                                     scale=diso[:, t:t + 1])
                for h in range(2):
                    nc.tensor.matmul(
                        u[h][:, t * 128:(t + 1) * 128],
                        xob[:, t * D + h * 128:t * D + (h + 1) * 128],
                        eye_s[:, :],
                        start=False, stop=(t == TS - 1),
                        skip_group_check=True)

            # ---- phase 4: scale columns by own dis, cast to bf16 ----
            for h in range(2):
                nc.vector.tensor_mul(y2[h][:, :], u[h][:, :], disrep[:, :])

            # ---- phase 5: out^T = W^T @ (scaled U^T) ----
            o = [psuo.tile([128, R], f32, name=f"o_{i}", tag="uo") for i in range(2)]
            for mh in range(2):
                for nk in range(2):
                    for s in range(2):
                        nc.tensor.matmul(
                            o[mh][:, s * 512:(s + 1) * 512],
                            Wb[:, nk * D + mh * 128:nk * D + (mh + 1) * 128],
                            y2[nk][:, s * 512:(s + 1) * 512],
                            start=(nk == 0), stop=(nk == 1),
                            skip_group_check=True)

            # ---- phase 6: relu(out^T + b), write transposed output ----
            for mh in range(2):
                nc.scalar.activation(
                    outsb[mh][:, :], o[mh][:, :], AF.Relu,
                    bias=bsb[:, mh:mh + 1], scale=1.0)
                nc.sync.dma_start(
                    out=outT.ap()[mh * 128:(mh + 1) * 128, :],
                    in_=outsb[mh][:, :])

    nc.compile()
    return nc


def _get_nc():
    if "nc" not in _CACHE:
        _CACHE["nc"] = _build_nc()
    return _CACHE["nc"]


def kernel(x, adj, W, b):
    import ml_dtypes
    from concourse.bass_utils import run_bass_kernel_spmd

    bf = ml_dtypes.bfloat16
    x = np.asarray(x, dtype=np.float32)
    adj = np.asarray(adj, dtype=np.float32)
    W = np.ascontiguousarray(np.asarray(W, dtype=np.float32)).astype(bf)
    b = np.ascontiguousarray(np.asarray(b, dtype=np.float32))

    nc = _get_nc()

    x_bf = np.ascontiguousarray(x).astype(bf)
    eye_np = np.eye(128, dtype=bf)
    eyef_np = np.eye(128, dtype=np.float32)
    in_maps = []
    for c in range(NCORES):
        rows = slice(c * R, (c + 1) * R)
        in_maps.append({
            "adjT": np.ascontiguousarray(adj[rows, :].T).astype(bf),
            "x": x_bf,
            "xown": x_bf[rows, :].copy(),
            "W": W,
            "b": b,
            "eye": eye_np,
            "eyef": eyef_np,
        })

    res = run_bass_kernel_spmd(nc, in_maps, core_ids=list(range(NCORES)))
    out = np.concatenate(
        [np.asarray(res.results[c]["outT"]).T for c in range(NCORES)], axis=0)
    return np.ascontiguousarray(out, dtype=np.float32)


if __name__ == "__main__":
    rng = np.random.default_rng(0)
    x = rng.standard_normal((N, D)).astype(np.float32)
    adj = rng.random((N, N)).astype(np.float32)
    W = rng.standard_normal((D, D)).astype(np.float32) * 0.06
    b = rng.standard_normal((D,)).astype(np.float32) * 0.06
    out = kernel(x=x, adj=adj, W=W, b=b)
    print(out.shape, out.dtype)


# revision 12
# speedup vs baseline: 1.0149x; 1.0149x over previous
"""GCN layer on 8 Trainium2 NeuronCores.

Computes relu(D^-1/2 (A+I) D^-1/2 X W + b) for N=8192, d=256.

Sharding: row-shard adj over N across the 8 cores (1024 rows each); x, W, b
replicated. Each core's shard is laid out column-major (adj[rows].T) and in
bf16 (the tensor-engine compute precision) so the contraction dim (adj
columns j) lands on SBUF partitions, which the PE matmul requires.

Pipeline per core (single NEFF):
  1. Stream the 16MB shard once (HWDGE) into a persistent SBUF cache, in two
     stages: first the columns for its rows 0:512 ("A"), then rows 512:1024
     ("B"). The tensor engine reduces row sums (matmul vs ones) as tiles land.
  2. AllGather #1 ships the A-half degrees while the B-half still streams;
     AllGather #2 ships the B-half. Degrees come back through a natural
     (contiguous) DMA + PE transpose into per-partition D^-1/2 tables.
  3. U^T = ((A+I) y)^T with y = D^-1/2 x: y is scaled chunk-by-chunk
     (alternating Scalar/Vector engines) just ahead of the matmuls; the
     matmuls for row-half A start after AllGather #1, hiding AllGather #2;
     +I enters via identity-matmuls of the core's own y rows.
  4. Scale by own D^-1/2 (free-dim broadcast), apply W, bias, ReLU, and
     write the output block transposed; the host stitches the 8 blocks.
"""

import numpy as np

N = 8192
D = 256
NCORES = 8
R = N // NCORES  # rows per core = 1024
KT = N // 128  # 64 j-tiles
TS = R // 128  # 8 own-row tiles

_CACHE = {}


def _build_nc():
    import concourse.bacc as bacc
    import concourse.tile as tile
    import concourse.mybir as mybir

    f32 = mybir.dt.float32
    bf16 = mybir.dt.bfloat16
    AF = mybir.ActivationFunctionType

    nc = bacc.Bacc("TRN2", target_bir_lowering=False, debug=False,
                   num_devices=NCORES)

    adjT = nc.dram_tensor("adjT", [N, R], bf16, kind="ExternalInput")
    xin = nc.dram_tensor("x", [N, D], bf16, kind="ExternalInput")
    xown = nc.dram_tensor("xown", [R, D], bf16, kind="ExternalInput")
    Win = nc.dram_tensor("W", [D, D], bf16, kind="ExternalInput")
    bin_ = nc.dram_tensor("b", [D], f32, kind="ExternalInput")
    eyeb = nc.dram_tensor("eye", [128, 128], bf16, kind="ExternalInput")
    eyef = nc.dram_tensor("eyef", [128, 128], f32, kind="ExternalInput")
    outT = nc.dram_tensor("outT", [D, R], f32, kind="ExternalOutput")

    with tile.TileContext(nc) as tc:
        from contextlib import ExitStack

        with ExitStack() as ctx:
            pp = ctx.enter_context(tc.tile_pool(name="persist", bufs=1))
            dp = ctx.enter_context(tc.tile_pool(name="dram", bufs=1, space="DRAM"))

            # ---- persistent SBUF tensors ----
            adjTb = pp.tile([128, KT * R], bf16)   # 128KB/partition cache
            xb = pp.tile([128, KT * D], bf16)      # x, partition = j%128
            xob = pp.tile([128, TS * D], bf16)     # own x rows
            Wb = pp.tile([128, 2 * D], bf16)       # W, partition = n%128
            bsb = pp.tile([128, 2], f32)           # bias, partition = m%128
            eye_s = pp.tile([128, 128], bf16)
            eyef_s = pp.tile([128, 128], f32)
            ones_bf = pp.tile([128, 1], bf16)
            deg_s = pp.tile([1, R], f32)           # local degree (+1), A|B halves
            disl = pp.tile([1, R], f32)            # local D^-1/2
            degn = pp.tile([64, 128], f32)         # gathered degrees, natural
            degln = pp.tile([8, 128], f32)         # local degrees, natural
            dis_pp = pp.tile([128, 64], f32)       # D^-1/2, partition = j%128
            diso = pp.tile([128, TS], f32)         # own D^-1/2, partition = i%128
            disrep = pp.tile([128, R], f32)        # own D^-1/2 on free dim
            y2 = [pp.tile([128, R], bf16, name=f"y2_{i}") for i in range(2)]
            outsb = [pp.tile([128, R], f32, name=f"outsb_{i}") for i in range(2)]

            degl_d = dp.tile([R], f32)
            dega_d = dp.tile([N], f32)
            disl_d = dp.tile([R], f32)

            nc.any.memset(ones_bf[:], 1.0)

            # ---- small loads first (sync HWDGE queue) ----
            nc.sync.dma_start(
                out=xob[:, :].rearrange("p (t n) -> p t n", n=D),
                in_=xown.ap().rearrange("(t p) n -> p t n", p=128))
            nc.sync.dma_start(
                out=Wb[:, :].rearrange("p (k m) -> p k m", m=D),
                in_=Win.ap().rearrange("(k p) m -> p k m", p=128))
            nc.sync.dma_start(
                out=bsb[:, :], in_=bin_.ap().rearrange("(h p) -> p h", p=128))
            nc.sync.dma_start(out=eye_s[:, :], in_=eyeb.ap())
            nc.sync.dma_start(out=eyef_s[:, :], in_=eyef.ap())

            # ---- phase 1: stream adjT + row sums ----
            GC = 16  # j-tiles per DMA chunk
            adjTb3 = adjTb[:, :].rearrange("p (k i) -> p k i", i=R)
            xb3 = xb[:, :].rearrange("p (k n) -> p k n", n=D)
            xin3 = xin.ap().rearrange("(k p) n -> p k n", p=128)

            for g in range(KT // GC):
                src = adjT.ap()[g * GC * 128:(g + 1) * GC * 128,
                                :].rearrange("(k p) i -> p k i", p=128)
                nc.sync.dma_start(
                    out=adjTb3[:, g * GC:(g + 1) * GC, :], in_=src)
            nc.sync.dma_start(out=xb3[:, :, :], in_=xin3[:, :, :])

            pdeg = ctx.enter_context(tc.tile_pool(name="psdeg", bufs=1, space="PSUM"))
            pst = ctx.enter_context(tc.tile_pool(name="pst", bufs=1, space="PSUM"))
            psuo = ctx.enter_context(tc.tile_pool(name="psuo", bufs=2, space="PSUM"))

            dps = pdeg.tile([1, 1024], f32, padded_shape=[128, 1024])
            for s in range(2):
                for k in range(KT):
                    nc.tensor.matmul(
                        dps[:, s * 512:(s + 1) * 512], ones_bf[:, :],
                        adjTb[:, k * R + s * 512:k * R + (s + 1) * 512],
                        start=(k == 0), stop=(k == KT - 1),
                        skip_group_check=True)
                # deg = rowsum + 1 (the +I term)
                nc.vector.tensor_scalar_add(
                    deg_s[:, s * 512:(s + 1) * 512],
                    dps[:, s * 512:(s + 1) * 512], 1.0)
            nc.scalar.dma_start(out=degl_d[:], in_=deg_s[0:1, :])
            nc.gpsimd.collective_compute(
                "AllGather", mybir.AluOpType.bypass,
                replica_groups=[list(range(NCORES))],
                ins=[degl_d.opt()], outs=[dega_d.opt()])

            # local dis for the free-dim broadcast (via DRAM round trip)
            nc.vector.reciprocal_approx_fast(disl[:, :], deg_s[:, :])
            nc.scalar.activation(disl[:, :], disl[:, :], AF.Sqrt)
            nc.scalar.dma_start(out=disl_d[:], in_=disl[0:1, :])
            nc.scalar.dma_start(
                out=disrep[:, :],
                in_=disl_d.opt().unsqueeze(0).partition_broadcast(128))

            # gathered degrees -> per-partition D^-1/2 via PE transpose
            tall = pst.tile([128, 72], f32)
            nc.scalar.dma_start(
                out=degn[:, :], in_=dega_d.opt().rearrange("(c f) -> c f", f=128))
            nc.tensor.transpose(tall[:, 0:64], degn[:, :], eyef_s[0:64, 0:64])
            nc.vector.reciprocal_approx_fast(dis_pp[:, :], tall[:, 0:64])
            nc.scalar.activation(dis_pp[:, :], dis_pp[:, :], AF.Sqrt)
            # local degrees -> own D^-1/2 table (for the +I rows)
            nc.scalar.dma_start(
                out=degln[:, :], in_=degl_d.opt().rearrange("(c f) -> c f", f=128))
            nc.tensor.transpose(tall[:, 64:72], degln[:, :], eyef_s[0:8, 0:8])
            nc.vector.reciprocal_approx_fast(diso[:, :], tall[:, 64:72])
            nc.scalar.activation(diso[:, :], diso[:, :], AF.Sqrt)

            # ---- phase 3: y = dis*x and U^T = ((A+I) y)^T ----
            u = [psuo.tile([128, R], f32, name=f"u_{i}", tag="uo") for i in range(2)]

            def dis_col(k):
                return dis_pp[:, k:k + 1]

            def scale_y(k):
                chunk = xb[:, k * D:(k + 1) * D]
                if k % 2 == 0:
                    nc.scalar.activation(chunk, chunk, AF.Copy,
                                         scale=dis_col(k))
                else:
                    nc.vector.tensor_scalar_mul(chunk, chunk, dis_col(k))

            for k in range(KT):
                scale_y(k)
            for k in range(KT):
                for h in range(2):
                    for s in range(2):
                        nc.tensor.matmul(
                            u[h][:, s * 512:(s + 1) * 512],
                            xb[:, k * D + h * 128:k * D + (h + 1) * 128],
                            adjTb[:, k * R + s * 512:k * R + (s + 1) * 512],
                            start=(k == 0), stop=False,
                            skip_group_check=True)
            # +I: U^T[n, own block t] += y_own[t]^T
            for t in range(TS):
                chunk = xob[:, t * D:(t + 1) * D]
                nc.scalar.activation(chunk, chunk, AF.Copy,
                                     scale=diso[:, t:t + 1])
                for h in range(2):
                    nc.tensor.matmul(
                        u[h][:, t * 128:(t + 1) * 128],
                        xob[:, t * D + h * 128:t * D + (h + 1) * 128],
                        eye_s[:, :],
                        start=False, stop=(t == TS - 1),
                        skip_group_check=True)

            # ---- phase 4: scale columns by own dis, cast to bf16 ----
            for h in range(2):
                nc.vector.tensor_mul(y2[h][:, :], u[h][:, :], disrep[:, :])

            # ---- phase 5: out^T = W^T @ (scaled U^T) ----
            o = [psuo.tile([128, R], f32, name=f"o_{i}", tag="uo") for i in range(2)]
            for mh in range(2):
                for nk in range(2):
                    for s in range(2):
                        nc.tensor.matmul(
                            o[mh][:, s * 512:(s + 1) * 512],
                            Wb[:, nk * D + mh * 128:nk * D + (mh + 1) * 128],
                            y2[nk][:, s * 512:(s + 1) * 512],
                            start=(nk == 0), stop=(nk == 1),
                            skip_group_check=True)

            # ---- phase 6: relu(out^T + b), write transposed output ----
            for mh in range(2):
                nc.scalar.activation(
                    outsb[mh][:, :], o[mh][:, :], AF.Relu,
                    bias=bsb[:, mh:mh + 1], scale=1.0)
                nc.sync.dma_start(
                    out=outT.ap()[mh * 128:(mh + 1) * 128, :],
                    in_=outsb[mh][:, :])

    nc.compile()
    return nc


def _get_nc():
    if "nc" not in _CACHE:
        _CACHE["nc"] = _build_nc()
    return _CACHE["nc"]


def kernel(x, adj, W, b):
    import ml_dtypes
    from concourse.bass_utils import run_bass_kernel_spmd

    bf = ml_dtypes.bfloat16
    x = np.asarray(x, dtype=np.float32)
    adj = np.asarray(adj, dtype=np.float32)
    W = np.ascontiguousarray(np.asarray(W, dtype=np.float32)).astype(bf)
    b = np.ascontiguousarray(np.asarray(b, dtype=np.float32))

    nc = _get_nc()

    x_bf = np.ascontiguousarray(x).astype(bf)
    eye_np = np.eye(128, dtype=bf)
    eyef_np = np.eye(128, dtype=np.float32)
    in_maps = []
    for c in range(NCORES):
        rows = slice(c * R, (c + 1) * R)
        in_maps.append({
            "adjT": np.ascontiguousarray(adj[rows, :].T).astype(bf),
            "x": x_bf,
            "xown": x_bf[rows, :].copy(),
            "W": W,
            "b": b,
            "eye": eye_np,
            "eyef": eyef_np,
        })

    res = run_bass_kernel_spmd(nc, in_maps, core_ids=list(range(NCORES)))
    out = np.concatenate(
        [np.asarray(res.results[c]["outT"]).T for c in range(NCORES)], axis=0)
    return np.ascontiguousarray(out, dtype=np.float32)


if __name__ == "__main__":
    rng = np.random.default_rng(0)
    x = rng.standard_normal((N, D)).astype(np.float32)
    adj = rng.random((N, N)).astype(np.float32)
    W = rng.standard_normal((D, D)).astype(np.float32) * 0.06
    b = rng.standard_normal((D,)).astype(np.float32) * 0.06
    out = kernel(x=x, adj=adj, W=W, b=b)
    print(out.shape, out.dtype)


# revision 13
# speedup vs baseline: 1.0165x; 1.0016x over previous
"""GCN layer on 8 Trainium2 NeuronCores.

Computes relu(D^-1/2 (A+I) D^-1/2 X W + b) for N=8192, d=256.

Sharding: row-shard adj over N across the 8 cores (1024 rows each); x, W, b
replicated. Each core's shard is laid out column-major (adj[rows].T) and in
bf16 (the tensor-engine compute precision) so the contraction dim (adj
columns j) lands on SBUF partitions, which the PE matmul requires.

Pipeline per core (single NEFF):
  1. Stream the 16MB shard once (HWDGE) into a persistent SBUF cache, in two
     stages: first the columns for its rows 0:512 ("A"), then rows 512:1024
     ("B"). The tensor engine reduces row sums (matmul vs ones) as tiles land.
  2. AllGather #1 ships the A-half degrees while the B-half still streams;
     AllGather #2 ships the B-half. Degrees come back through a natural
     (contiguous) DMA + PE transpose into per-partition D^-1/2 tables.
  3. U^T = ((A+I) y)^T with y = D^-1/2 x: y is scaled chunk-by-chunk
     (alternating Scalar/Vector engines) just ahead of the matmuls; the
     matmuls for row-half A start after AllGather #1, hiding AllGather #2;
     +I enters via identity-matmuls of the core's own y rows.
  4. Scale by own D^-1/2 (free-dim broadcast), apply W, bias, ReLU, and
     write the output block transposed; the host stitches the 8 blocks.
"""

import numpy as np

N = 8192
D = 256
NCORES = 8
R = N // NCORES  # rows per core = 1024
KT = N // 128  # 64 j-tiles
TS = R // 128  # 8 own-row tiles

_CACHE = {}


def _build_nc():
    import concourse.bacc as bacc
    import concourse.tile as tile
    import concourse.mybir as mybir

    f32 = mybir.dt.float32
    bf16 = mybir.dt.bfloat16
    AF = mybir.ActivationFunctionType

    nc = bacc.Bacc("TRN2", target_bir_lowering=False, debug=False,
                   num_devices=NCORES)

    adjT = nc.dram_tensor("adjT", [N, R], bf16, kind="ExternalInput")
    xin = nc.dram_tensor("x", [N, D], bf16, kind="ExternalInput")
    xown = nc.dram_tensor("xown", [R, D], bf16, kind="ExternalInput")
    Win = nc.dram_tensor("W", [D, D], bf16, kind="ExternalInput")
    bin_ = nc.dram_tensor("b", [D], f32, kind="ExternalInput")
    eyeb = nc.dram_tensor("eye", [128, 128], bf16, kind="ExternalInput")
    eyef = nc.dram_tensor("eyef", [128, 128], f32, kind="ExternalInput")
    outT = nc.dram_tensor("outT", [D, R], f32, kind="ExternalOutput")

    with tile.TileContext(nc) as tc:
        from contextlib import ExitStack

        with ExitStack() as ctx:
            pp = ctx.enter_context(tc.tile_pool(name="persist", bufs=1))
            dp = ctx.enter_context(tc.tile_pool(name="dram", bufs=1, space="DRAM"))

            # ---- persistent SBUF tensors ----
            adjTb = pp.tile([128, KT * R], bf16)   # 128KB/partition cache
            xb = pp.tile([128, KT * D], bf16)      # x, partition = j%128
            xob = pp.tile([128, TS * D], bf16)     # own x rows
            Wb = pp.tile([128, 2 * D], bf16)       # W, partition = n%128
            bsb = pp.tile([128, 2], f32)           # bias, partition = m%128
            eye_s = pp.tile([128, 128], bf16)
            eyef_s = pp.tile([128, 128], f32)
            ones_bf = pp.tile([128, 1], bf16)
            deg_s = pp.tile([1, R], f32)           # local degree (+1), A|B halves
            disl = pp.tile([1, R], f32)            # local D^-1/2
            degnA = pp.tile([32, 128], f32)        # gathered degrees, natural
            degnB = pp.tile([32, 128], f32)
            deglnA = pp.tile([4, 128], f32)        # local degrees, natural
            deglnB = pp.tile([4, 128], f32)
            disA = pp.tile([128, 32], f32)         # D^-1/2 for j-tiles k%8<4
            disB = pp.tile([128, 32], f32)         # D^-1/2 for j-tiles k%8>=4
            diso = pp.tile([128, TS], f32)         # own D^-1/2, partition = i%128
            disrep = pp.tile([128, R], f32)        # own D^-1/2 on free dim
            y2 = [pp.tile([128, R], bf16, name=f"y2_{i}") for i in range(2)]
            outsb = [pp.tile([128, R], f32, name=f"outsb_{i}") for i in range(2)]

            deglA_d = dp.tile([R // 2], f32)
            deglB_d = dp.tile([R // 2], f32)
            degaA_d = dp.tile([N // 2], f32)
            degaB_d = dp.tile([N // 2], f32)
            disl_d = dp.tile([R], f32)

            nc.any.memset(ones_bf[:], 1.0)

            # ---- small loads first (sync HWDGE queue) ----
            nc.sync.dma_start(
                out=xob[:, :].rearrange("p (t n) -> p t n", n=D),
                in_=xown.ap().rearrange("(t p) n -> p t n", p=128))
            nc.sync.dma_start(
                out=Wb[:, :].rearrange("p (k m) -> p k m", m=D),
                in_=Win.ap().rearrange("(k p) m -> p k m", p=128))
            nc.sync.dma_start(
                out=bsb[:, :], in_=bin_.ap().rearrange("(h p) -> p h", p=128))
            nc.sync.dma_start(out=eye_s[:, :], in_=eyeb.ap())
            nc.sync.dma_start(out=eyef_s[:, :], in_=eyef.ap())

            # ---- phase 1: stream adjT + row sums ----
            GC = 16  # j-tiles per DMA chunk
            adjTb3 = adjTb[:, :].rearrange("p (k i) -> p k i", i=R)
            xb3 = xb[:, :].rearrange("p (k n) -> p k n", n=D)
            xin3 = xin.ap().rearrange("(k p) n -> p k n", p=128)

            def stream_half(s):
                lo, hi = s * 512, (s + 1) * 512
                for g in range(KT // GC):
                    src = adjT.ap()[g * GC * 128:(g + 1) * GC * 128,
                                    lo:hi].rearrange("(k p) i -> p k i", p=128)
                    nc.sync.dma_start(
                        out=adjTb3[:, g * GC:(g + 1) * GC, lo:hi], in_=src)

            stream_half(0)                       # rows A of all cores
            stream_half(1)                       # rows B
            nc.sync.dma_start(out=xb3[:, :, :], in_=xin3[:, :, :])

            pdeg = ctx.enter_context(tc.tile_pool(name="psdeg", bufs=1, space="PSUM"))
            pst = ctx.enter_context(tc.tile_pool(name="pst", bufs=1, space="PSUM"))
            psuo = ctx.enter_context(tc.tile_pool(name="psuo", bufs=2, space="PSUM"))

            dps = pdeg.tile([1, 1024], f32, padded_shape=[128, 1024])
            degl_halves = [deglA_d, deglB_d]
            dega_halves = [degaA_d, degaB_d]
            for s in range(2):
                for k in range(KT):
                    nc.tensor.matmul(
                        dps[:, s * 512:(s + 1) * 512], ones_bf[:, :],
                        adjTb[:, k * R + s * 512:k * R + (s + 1) * 512],
                        start=(k == 0), stop=(k == KT - 1),
                        skip_group_check=True)
                # deg = rowsum + 1 (the +I term)
                nc.vector.tensor_scalar_add(
                    deg_s[:, s * 512:(s + 1) * 512],
                    dps[:, s * 512:(s + 1) * 512], 1.0)
                nc.scalar.dma_start(out=degl_halves[s][:],
                                    in_=deg_s[0:1, s * 512:(s + 1) * 512])
                nc.gpsimd.collective_compute(
                    "AllGather", mybir.AluOpType.bypass,
                    replica_groups=[list(range(NCORES))],
                    ins=[degl_halves[s].opt()], outs=[dega_halves[s].opt()])

            # local dis for the free-dim broadcast (via DRAM round trip)
            nc.vector.reciprocal_approx_fast(disl[:, :], deg_s[:, :])
            nc.scalar.activation(disl[:, :], disl[:, :], AF.Sqrt)
            nc.scalar.dma_start(out=disl_d[:], in_=disl[0:1, :])
            nc.scalar.dma_start(
                out=disrep[:, :],
                in_=disl_d.opt().unsqueeze(0).partition_broadcast(128))

            # gathered degrees -> per-partition D^-1/2 via PE transpose
            tall = pst.tile([128, 72], f32)
            for s in range(2):
                degn_s = [degnA, degnB][s]
                dis_s = [disA, disB][s]
                nc.scalar.dma_start(
                    out=degn_s[:, :],
                    in_=dega_halves[s].opt().rearrange("(c f) -> c f", f=128))
                nc.tensor.transpose(tall[:, s * 32:(s + 1) * 32], degn_s[:, :],
                                    eyef_s[0:32, 0:32])
                nc.vector.reciprocal_approx_fast(
                    dis_s[:, :], tall[:, s * 32:(s + 1) * 32])
                nc.scalar.activation(dis_s[:, :], dis_s[:, :], AF.Sqrt)
                # local degrees -> own D^-1/2 table (for the +I rows)
                degln_s = [deglnA, deglnB][s]
                nc.scalar.dma_start(
                    out=degln_s[:, :],
                    in_=degl_halves[s].opt().rearrange("(c f) -> c f", f=128))
                nc.tensor.transpose(tall[:, 64 + s * 4:68 + s * 4],
                                    degln_s[:, :], eyef_s[0:4, 0:4])
                nc.vector.reciprocal_approx_fast(
                    diso[:, s * 4:(s + 1) * 4], tall[:, 64 + s * 4:68 + s * 4])
                nc.scalar.activation(diso[:, s * 4:(s + 1) * 4],
                                     diso[:, s * 4:(s + 1) * 4], AF.Sqrt)

            # ---- phase 3: y = dis*x and U^T = ((A+I) y)^T ----
            u = [psuo.tile([128, R], f32, name=f"u_{i}", tag="uo") for i in range(2)]

            def dis_col(k):
                c, t = divmod(k, 8)
                if t < 4:
                    return disA[:, 4 * c + t:4 * c + t + 1]
                return disB[:, 4 * c + t - 4:4 * c + t - 3]

            def scale_y(k):
                chunk = xb[:, k * D:(k + 1) * D]
                if k % 2 == 0:
                    nc.scalar.activation(chunk, chunk, AF.Copy,
                                         scale=dis_col(k))
                else:
                    nc.vector.tensor_scalar_mul(chunk, chunk, dis_col(k))

            ksA = [k for k in range(KT) if k % 8 < 4]
            ksB = [k for k in range(KT) if k % 8 >= 4]
            for k in ksA:
                scale_y(k)
            for s in range(2):
                for k in ksA:
                    for h in range(2):
                        nc.tensor.matmul(
                            u[h][:, s * 512:(s + 1) * 512],
                            xb[:, k * D + h * 128:k * D + (h + 1) * 128],
                            adjTb[:, k * R + s * 512:k * R + (s + 1) * 512],
                            start=(k == ksA[0]), stop=False,
                            skip_group_check=True)
            # +I: U^T[n, own block t] += y_own[t]^T
            for t in range(TS):
                chunk = xob[:, t * D:(t + 1) * D]
                nc.scalar.activation(chunk, chunk, AF.Copy,
                                     scale=diso[:, t:t + 1])
                for h in range(2):
                    nc.tensor.matmul(
                        u[h][:, t * 128:(t + 1) * 128],
                        xob[:, t * D + h * 128:t * D + (h + 1) * 128],
                        eye_s[:, :],
                        start=False, stop=False,
                        skip_group_check=True)
            for k in ksB:
                scale_y(k)
            for k in ksB:
                for h in range(2):
                    for s in range(2):
                        nc.tensor.matmul(
                            u[h][:, s * 512:(s + 1) * 512],
                            xb[:, k * D + h * 128:k * D + (h + 1) * 128],
                            adjTb[:, k * R + s * 512:k * R + (s + 1) * 512],
                            start=False, stop=(k == ksB[-1]),
                            skip_group_check=True)

            # ---- phase 4: scale columns by own dis, cast to bf16 ----
            for h in range(2):
                nc.vector.tensor_mul(y2[h][:, :], u[h][:, :], disrep[:, :])

            # ---- phase 5: out^T = W^T @ (scaled U^T) ----
            o = [psuo.tile([128, R], f32, name=f"o_{i}", tag="uo") for i in range(2)]
            for mh in range(2):
                for nk in range(2):
                    for s in range(2):
                        nc.tensor.matmul(
                            o[mh][:, s * 512:(s + 1) * 512],
                            Wb[:, nk * D + mh * 128:nk * D + (mh + 1) * 128],
                            y2[nk][:, s * 512:(s + 1) * 512],
                            start=(nk == 0), stop=(nk == 1),
                            skip_group_check=True)

            # ---- phase 6: relu(out^T + b), write transposed output ----
            for mh in range(2):
                nc.scalar.activation(
                    outsb[mh][:, :], o[mh][:, :], AF.Relu,
                    bias=bsb[:, mh:mh + 1], scale=1.0)
                nc.sync.dma_start(
                    out=outT.ap()[mh * 128:(mh + 1) * 128, :],
                    in_=outsb[mh][:, :])

    nc.compile()
    return nc


def _get_nc():
    if "nc" not in _CACHE:
        _CACHE["nc"] = _build_nc()
    return _CACHE["nc"]


def kernel(x, adj, W, b):
    import ml_dtypes
    from concourse.bass_utils import run_bass_kernel_spmd

    bf = ml_dtypes.bfloat16
    x = np.asarray(x, dtype=np.float32)
    adj = np.asarray(adj, dtype=np.float32)
    W = np.ascontiguousarray(np.asarray(W, dtype=np.float32)).astype(bf)
    b = np.ascontiguousarray(np.asarray(b, dtype=np.float32))

    nc = _get_nc()

    x_bf = np.ascontiguousarray(x).astype(bf)
    eye_np = np.eye(128, dtype=bf)
    eyef_np = np.eye(128, dtype=np.float32)
    in_maps = []
    for c in range(NCORES):
        rows = slice(c * R, (c + 1) * R)
        in_maps.append({
            "adjT": np.ascontiguousarray(adj[rows, :].T).astype(bf),
            "x": x_bf,
            "xown": x_bf[rows, :].copy(),
            "W": W,
            "b": b,
            "eye": eye_np,
            "eyef": eyef_np,
        })

    res = run_bass_kernel_spmd(nc, in_maps, core_ids=list(range(NCORES)))
    out = np.concatenate(
        [np.asarray(res.results[c]["outT"]).T for c in range(NCORES)], axis=0)
    return np.ascontiguousarray(out, dtype=np.float32)


if __name__ == "__main__":
    rng = np.random.default_rng(0)
    x = rng.standard_normal((N, D)).astype(np.float32)
    adj = rng.random((N, N)).astype(np.float32)
    W = rng.standard_normal((D, D)).astype(np.float32) * 0.06
    b = rng.standard_normal((D,)).astype(np.float32) * 0.06
    out = kernel(x=x, adj=adj, W=W, b=b)
    print(out.shape, out.dtype)


# revision 14
# speedup vs baseline: 1.0391x; 1.0222x over previous
"""GCN layer on 8 Trainium2 NeuronCores.

Computes relu(D^-1/2 (A+I) D^-1/2 X W + b) for N=8192, d=256.

Sharding: row-shard adj over N across the 8 cores (1024 rows each); x, W, b
replicated. Each core's adj shard is uploaded as the bf16 SBUF image it will
occupy on chip: partition p holds adj[1024c+i, 128k+p] at column k*1024+i,
i.e. the contraction dim j sits on partitions (as the PE matmul needs) and
every partition's data is one contiguous DRAM run (full DMA line rate).

Pipeline per core (single NEFF):
  1. Stream the 16MB shard once (HWDGE, 2MB slices) into the persistent SBUF
     cache; the tensor engine reduces row sums (matmul vs ones) as slices
     land.
  2. One AllGather ships the 8 local degree vectors (4KB each); degrees come
     back through a natural (contiguous) DMA + PE transpose into
     per-partition D^-1/2 tables.
  3. U^T = ((A+I) y)^T with y = D^-1/2 x: x chunks are scaled in place
     (Scalar/Vector engines alternating, all ahead of the matmuls), then 256
     accumulating matmuls run from SBUF; +I enters via identity-matmuls of
     the core's own y rows.
  4. Scale by own D^-1/2 (free-dim broadcast via a DMA broadcast round trip),
     apply W, bias, ReLU, and write the output block transposed; the host
     stitches the 8 blocks.
"""

import numpy as np

N = 8192
D = 256
NCORES = 8
R = N // NCORES  # rows per core = 1024
KT = N // 128  # 64 j-tiles
TS = R // 128  # 8 own-row tiles

_CACHE = {}


def _build_nc():
    import concourse.bacc as bacc
    import concourse.tile as tile
    import concourse.mybir as mybir

    f32 = mybir.dt.float32
    bf16 = mybir.dt.bfloat16
    AF = mybir.ActivationFunctionType

    nc = bacc.Bacc("TRN2", target_bir_lowering=False, debug=False,
                   num_devices=NCORES)

    adjS = nc.dram_tensor("adjS", [128, KT * R], bf16, kind="ExternalInput")
    xS = nc.dram_tensor("xS", [128, KT * D], bf16, kind="ExternalInput")
    xoS = nc.dram_tensor("xoS", [128, TS * D], bf16, kind="ExternalInput")
    Win = nc.dram_tensor("W", [D, D], bf16, kind="ExternalInput")
    bin_ = nc.dram_tensor("b", [D], f32, kind="ExternalInput")
    eyeb = nc.dram_tensor("eye", [128, 128], bf16, kind="ExternalInput")
    eyef = nc.dram_tensor("eyef", [128, 128], f32, kind="ExternalInput")
    outT = nc.dram_tensor("outT", [D, R], f32, kind="ExternalOutput")

    with tile.TileContext(nc) as tc:
        from contextlib import ExitStack

        with ExitStack() as ctx:
            pp = ctx.enter_context(tc.tile_pool(name="persist", bufs=1))
            dp = ctx.enter_context(tc.tile_pool(name="dram", bufs=1, space="DRAM"))

            # ---- persistent SBUF tensors ----
            adjTb = pp.tile([128, KT * R], bf16)   # 128KB/partition cache
            xb = pp.tile([128, KT * D], bf16)      # x, partition = j%128
            xob = pp.tile([128, TS * D], bf16)     # own x rows
            Wb = pp.tile([128, 2 * D], bf16)       # W, partition = n%128
            bsb = pp.tile([128, 2], f32)           # bias, partition = m%128
            eye_s = pp.tile([128, 128], bf16)
            eyef_s = pp.tile([128, 128], f32)
            ones_bf = pp.tile([128, 1], bf16)
            deg_s = pp.tile([1, R], f32)           # local degree (+1)
            disl = pp.tile([1, R], f32)            # local D^-1/2
            degn = pp.tile([64, 128], f32)         # gathered degrees, natural
            degln = pp.tile([8, 128], f32)         # local degrees, natural
            dis_pp = pp.tile([128, KT], f32)       # D^-1/2, partition = j%128
            diso = pp.tile([128, TS], f32)         # own D^-1/2, partition = i%128
            disrep = pp.tile([128, R], f32)        # own D^-1/2 on free dim
            y2 = [pp.tile([128, R], bf16, name=f"y2_{i}") for i in range(2)]
            outsb = [pp.tile([128, R], f32, name=f"outsb_{i}") for i in range(2)]

            degl_d = dp.tile([R], f32)
            dega_d = dp.tile([N], f32)
            disl_d = dp.tile([R], f32)

            nc.any.memset(ones_bf[:], 1.0)

            # ---- small loads first (sync HWDGE queue) ----
            nc.sync.dma_start(out=xob[:, :], in_=xoS.ap())
            nc.sync.dma_start(
                out=Wb[:, :].rearrange("p (k m) -> p k m", m=D),
                in_=Win.ap().rearrange("(k p) m -> p k m", p=128))
            nc.sync.dma_start(
                out=bsb[:, :], in_=bin_.ap().rearrange("(h p) -> p h", p=128))
            nc.sync.dma_start(out=eye_s[:, :], in_=eyeb.ap())
            nc.sync.dma_start(out=eyef_s[:, :], in_=eyef.ap())

            # ---- phase 1: stream the SBUF image + row sums on PE ----
            GC = 8  # j-tiles per DMA slice (2MB each, 16KB/partition runs)
            for g in range(KT // GC):
                c0, c1 = g * GC * R, (g + 1) * GC * R
                nc.sync.dma_start(out=adjTb[:, c0:c1], in_=adjS.ap()[:, c0:c1])
            nc.sync.dma_start(out=xb[:, :], in_=xS.ap())

            pdeg = ctx.enter_context(
                tc.tile_pool(name="psdeg", bufs=1, space="PSUM"))
            pst = ctx.enter_context(
                tc.tile_pool(name="pst", bufs=1, space="PSUM"))
            psuo = ctx.enter_context(
                tc.tile_pool(name="psuo", bufs=2, space="PSUM"))

            dps = pdeg.tile([1, 1024], f32, padded_shape=[128, 1024])
            for s in range(2):
                for k in range(KT):
                    nc.tensor.matmul(
                        dps[:, s * 512:(s + 1) * 512], ones_bf[:, :],
                        adjTb[:, k * R + s * 512:k * R + (s + 1) * 512],
                        start=(k == 0), stop=(k == KT - 1),
                        skip_group_check=True)
                # deg = rowsum + 1 (the +I term)
                nc.vector.tensor_scalar_add(
                    deg_s[:, s * 512:(s + 1) * 512],
                    dps[:, s * 512:(s + 1) * 512], 1.0)

            # ---- phase 2: AllGather degrees ----
            nc.scalar.dma_start(out=degl_d[:], in_=deg_s[0:1, :])
            nc.gpsimd.collective_compute(
                "AllGather", mybir.AluOpType.bypass,
                replica_groups=[list(range(NCORES))],
                ins=[degl_d.opt()], outs=[dega_d.opt()])

            # local dis for the free-dim broadcast (via DRAM round trip)
            nc.vector.reciprocal_approx_fast(disl[:, :], deg_s[:, :])
            nc.scalar.activation(disl[:, :], disl[:, :], AF.Sqrt)
            nc.scalar.dma_start(out=disl_d[:], in_=disl[0:1, :])
            nc.scalar.dma_start(
                out=disrep[:, :],
                in_=disl_d.opt().unsqueeze(0).partition_broadcast(128))

            # gathered degrees -> per-partition D^-1/2 via PE transpose
            tall = pst.tile([128, 72], f32)
            nc.scalar.dma_start(
                out=degn[:, :], in_=dega_d.opt().rearrange("(c f) -> c f", f=128))
            nc.tensor.transpose(tall[:, 0:64], degn[:, :], eyef_s[0:64, 0:64])
            nc.vector.reciprocal_approx_fast(dis_pp[:, :], tall[:, 0:64])
            nc.scalar.activation(dis_pp[:, :], dis_pp[:, :], AF.Sqrt)
            # local degrees -> own D^-1/2 table (for the +I rows)
            nc.scalar.dma_start(
                out=degln[:, :], in_=degl_d.opt().rearrange("(c f) -> c f", f=128))
            nc.tensor.transpose(tall[:, 64:72], degln[:, :], eyef_s[0:8, 0:8])
            nc.vector.reciprocal_approx_fast(diso[:, :], tall[:, 64:72])
            nc.scalar.activation(diso[:, :], diso[:, :], AF.Sqrt)

            # ---- phase 3: y = dis*x, then U^T = ((A+I) y)^T ----
            u = [psuo.tile([128, R], f32, name=f"u_{i}", tag="uo")
                 for i in range(2)]

            def scale_y(k):
                chunk = xb[:, k * D:(k + 1) * D]
                if k % 2 == 0:
                    nc.scalar.activation(chunk, chunk, AF.Copy,
                                         scale=dis_pp[:, k:k + 1])
                else:
                    nc.vector.tensor_scalar_mul(chunk, chunk,
                                                dis_pp[:, k:k + 1])

            for k in range(KT):
                scale_y(k)
            for k in range(KT):
                for h in range(2):
                    for s in range(2):
                        nc.tensor.matmul(
                            u[h][:, s * 512:(s + 1) * 512],
                            xb[:, k * D + h * 128:k * D + (h + 1) * 128],
                            adjTb[:, k * R + s * 512:k * R + (s + 1) * 512],
                            start=(k == 0), stop=False,
                            skip_group_check=True)
            # +I: U^T[n, own block t] += y_own[t]^T
            for t in range(TS):
                chunk = xob[:, t * D:(t + 1) * D]
                nc.scalar.activation(chunk, chunk, AF.Copy,
                                     scale=diso[:, t:t + 1])
                for h in range(2):
                    nc.tensor.matmul(
                        u[h][:, t * 128:(t + 1) * 128],
                        xob[:, t * D + h * 128:t * D + (h + 1) * 128],
                        eye_s[:, :],
                        start=False, stop=(t == TS - 1),
                        skip_group_check=True)

            # ---- phase 4: scale columns by own dis, cast to bf16 ----
            for h in range(2):
                nc.vector.tensor_mul(y2[h][:, :], u[h][:, :], disrep[:, :])

            # ---- phase 5: out^T = W^T @ (scaled U^T) ----
            o = [psuo.tile([128, R], f32, name=f"o_{i}", tag="uo")
                 for i in range(2)]
            for mh in range(2):
                for nk in range(2):
                    for s in range(2):
                        nc.tensor.matmul(
                            o[mh][:, s * 512:(s + 1) * 512],
                            Wb[:, nk * D + mh * 128:nk * D + (mh + 1) * 128],
                            y2[nk][:, s * 512:(s + 1) * 512],
                            start=(nk == 0), stop=(nk == 1),
                            skip_group_check=True)

            # ---- phase 6: relu(out^T + b), write transposed output ----
            for mh in range(2):
                nc.scalar.activation(
                    outsb[mh][:, :], o[mh][:, :], AF.Relu,
                    bias=bsb[:, mh:mh + 1], scale=1.0)
                nc.sync.dma_start(
                    out=outT.ap()[mh * 128:(mh + 1) * 128, :],
                    in_=outsb[mh][:, :])

    nc.compile()
    return nc


def _get_nc():
    if "nc" not in _CACHE:
        _CACHE["nc"] = _build_nc()
    return _CACHE["nc"]


def _sbuf_image(mat_bf16):
    """[T*128, F] -> [128, T*F] where partition p holds rows {128t+p}."""
    t128, f = mat_bf16.shape
    t = t128 // 128
    return np.ascontiguousarray(
        mat_bf16.reshape(t, 128, f).transpose(1, 0, 2).reshape(128, t * f))


def kernel(x, adj, W, b):
    import ml_dtypes
    from concourse.bass_utils import run_bass_kernel_spmd

    bf = ml_dtypes.bfloat16
    x = np.asarray(x, dtype=np.float32)
    adj = np.asarray(adj, dtype=np.float32)
    W = np.ascontiguousarray(np.asarray(W, dtype=np.float32)).astype(bf)
    b = np.ascontiguousarray(np.asarray(b, dtype=np.float32))

    nc = _get_nc()

    x_bf = np.ascontiguousarray(x).astype(bf)
    xS = _sbuf_image(x_bf)
    eye_np = np.eye(128, dtype=bf)
    eyef_np = np.eye(128, dtype=np.float32)
    in_maps = []
    for c in range(NCORES):
        rows = slice(c * R, (c + 1) * R)
        adjT_c = np.ascontiguousarray(adj[rows, :].T).astype(bf)
        in_maps.append({
            "adjS": _sbuf_image(adjT_c),
            "xS": xS,
            "xoS": _sbuf_image(x_bf[rows, :]),
            "W": W,
            "b": b,
            "eye": eye_np,
            "eyef": eyef_np,
        })

    res = run_bass_kernel_spmd(nc, in_maps, core_ids=list(range(NCORES)))
    out = np.concatenate(
        [np.asarray(res.results[c]["outT"]).T for c in range(NCORES)], axis=0)
    return np.ascontiguousarray(out, dtype=np.float32)


if __name__ == "__main__":
    rng = np.random.default_rng(0)
    x = rng.standard_normal((N, D)).astype(np.float32)
    adj = rng.random((N, N)).astype(np.float32)
    W = rng.standard_normal((D, D)).astype(np.float32) * 0.06
    b = rng.standard_normal((D,)).astype(np.float32) * 0.06
    out = kernel(x=x, adj=adj, W=W, b=b)
    print(out.shape, out.dtype)
